# revision 17
# baseline (speedup 1.0000x reference)
"""BrainAgeGAT Trainium2 kernel: 2-layer GATv2 + mean-pool + MLP on 8 NeuronCores.

v2 — slot-major edge layout (partition = destination slot, free dim = edge
rank), replacing v1's edge-major one-hot-scatter design:

  - Destination nodes are sharded by node id across 8 cores; each core's 6250
    nodes are DEGREE-SORTED and packed into 49 blocks of 128 slots. Block b
    processes a [128 slots, Db ranks] rectangle of edges (Db = max degree in
    the block, nearly uniform thanks to the degree sort).
  - xr[dst] needs NO gather/expansion: it is one row per partition,
    broadcast along the rank axis by a stride-0 AP (runs at full DVE 2x).
  - The segment softmax-sum is a free-dim halving tree (contiguous slices,
    DVE 2x) instead of one-hot matmuls — the TensorEngine is out of the
    per-edge path entirely.
  - xl[src] rows (512B bf16) are fetched with SWDGE dma_gather, <=1024 rows
    per call, round-robined over 4 SWDGE queues (queues drain in parallel;
    measured ~3.4ns/row aggregate vs ~10ns/row on one queue).
  - int16 gather indices cap the table at 32768 rows; the 50176-row global
    table is covered by TWO gathers (piece A rows 0..32767, piece B the
    rest), with off-piece edges fetching a guaranteed-zero row, merged by
    one add: xl = gA + gB.
  - Pad edges (rank >= degree) gather the zero row and get -60 added to
    their logits before exp (-> weight ~0).
  - Logits: TT mult by broadcast att (2x) + tensor_reduce over c (1x);
    exp on ACT reading the logit row broadcast along c.
"""

import sys

sys.path.insert(0, "/opt/trn_rl_repo")

import ml_dtypes
import numpy as np

import concourse.bacc as bacc
import concourse.bass as bass
import concourse.mybir as mybir
import concourse.tile as tile
from concourse import library_config  # noqa: F401  (side-effect config)
from concourse.vector_clock import ScopedClock

BF16 = ml_dtypes.bfloat16

# ---------------------------------------------------------------------------
# Patches for walrus' one-sync-wait-per-instruction limit (as v1).
# ---------------------------------------------------------------------------


def _drain_and_barrier(self, tick_clock, wait_clock):
    nc = self.nc
    probe = nc.sync.nop(nofuse=True, hint="drain_wait_split")
    wait_clock.add_sem_waits(probe.ins, ScopedClock({None: tick_clock.global_clock}))
    si = probe.ins.sync_info
    waits = list(si.on_wait) if si and si.on_wait else []
    if len(waits) > 1:
        si.on_wait = waits[:1]
        for w in waits[1:]:
            extra = nc.sync.nop(nofuse=True, hint="drain_wait_split")
            extra.ins.sync_info = type(si)(on_wait=[w], on_update=[])
    nc.sync.drain()
    nc.all_engine_barrier()
    assert self.sems is not None
    popped = nc._tile_sem_poison_stack.pop()
    assert popped is self._sem_poison
    nc.clear_and_free_semaphores(list(self.sems.allocated().values()))
    nc.all_engine_barrier()


tile.TileContext._drain_and_barrier = _drain_and_barrier


def _split_waits(nc):
    """walrus codegen accepts one sync-wait command per instruction; Tile can
    emit several. Hoist extras onto preceding same-engine NoOps."""
    for bb in nc.main_func.blocks:
        out = []
        for ins in bb.instructions:
            si = ins.sync_info
            waits = list(si.on_wait) if si and si.on_wait else []
            if len(waits) > 1:
                for w in waits[:-1]:
                    nop = mybir.InstNoOp(
                        name=nc.get_next_instruction_name(), ins=[], outs=[]
                    )
                    nop.engine = ins.engine
                    nop.sync_info = mybir.SyncInfo(on_wait=[w], on_update=[])
                    nc.register_instruction(nop)
                    out.append(nop)
                si.on_wait = [waits[-1]]
            out.append(ins)
        bb.instructions = out


# ---------------------------------------------------------------------------
# Model dimensions (hardcoded per problem spec)
# ---------------------------------------------------------------------------
N = 50000
E = 800000
G = 128
H = 8
C = 32
HC = H * C  # 256
P = 128
NCORES = 8
NPC = N // NCORES          # 6250
NBLK = (NPC + P - 1) // P  # 49
CAP = NBLK * P             # 6272
CAPEXT = NCORES * CAP      # 50176
PIECE = 32768              # gather piece boundary (int16 idx limit)
ZROW_A = NPC + 1           # core-0 tail slot: global row 6251 (zero row, piece A)
ZROW_B_ABS = 5 * CAP + NPC + 1  # core-5 tail slot: 37611 (zero row, piece B)
ZROW_B = ZROW_B_ABS - PIECE
DCH = 8                    # d-ranks per gather call (8*128 = 1024 rows, HW max)
NQ = 4                     # SWDGE queues
MASKNEG = -60.0


def _f32(a):
    return np.ascontiguousarray(a, dtype=np.float32)


def _bf(a):
    return np.ascontiguousarray(np.asarray(a, dtype=np.float32).astype(BF16))


def _wrap_idx(ids):
    """Gather-index list -> [128, len/16] int16 in the SWDGE wrap layout
    (idx j read from [j % 16, j // 16], replicated over the 8 Q7 cores)."""
    ids = np.asarray(ids, np.int16)
    assert len(ids) % 16 == 0
    w = ids.reshape(-1, 16).T  # [16, s]
    return np.tile(w, (8, 1))  # [128, s]


# ---------------------------------------------------------------------------
# Host-side planning
# ---------------------------------------------------------------------------


def _plan(edge_index):
    """Degree-sort nodes per core, assign (block, slot), bucket edges into
    [slot, rank] rectangles, build gather index + mask arrays."""
    src_all = np.concatenate([edge_index[0], np.arange(N)]).astype(np.int64)
    dst_all = np.concatenate([edge_index[1], np.arange(N)]).astype(np.int64)
    deg = np.bincount(dst_all, minlength=N)

    rowof = np.empty(N, np.int64)
    orders = []
    db_core = np.zeros((NCORES, NBLK), np.int64)
    for c in range(NCORES):
        own = np.arange(c * NPC, (c + 1) * NPC)
        order = own[np.argsort(-deg[own], kind="stable")]
        rowof[order] = c * CAP + np.arange(NPC)
        orders.append(order)
        ds = deg[order]
        for b in range(NBLK):
            db_core[c, b] = ds[b * P : min((b + 1) * P, NPC)].max()
    Db = db_core.max(axis=0)
    Db = np.maximum(Db + (Db % 2), 2).astype(int)  # even, >=2
    col0 = np.concatenate([[0], np.cumsum(Db)]).astype(int)  # in d units
    ncols = int(col0[-1])  # sum Db

    cores = []
    for c in range(NCORES):
        sel = (dst_all >= c * NPC) & (dst_all < (c + 1) * NPC)
        s = rowof[src_all[sel]]
        r = rowof[dst_all[sel]] - c * CAP
        o2 = np.argsort(r, kind="stable")
        s, r = s[o2], r[o2]
        starts = np.searchsorted(r, np.arange(NPC + 1))
        d = np.arange(len(r)) - starts[r]
        p_ = r % P
        b_ = r // P

        ixa = np.zeros((P, 8 * ncols), np.int16)
        ixb = np.zeros((P, 8 * ncols), np.int16)
        mk = np.full((P, ncols), MASKNEG, np.float32)
        for blk in range(NBLK):
            m_ = b_ == blk
            db = Db[blk]
            j = d[m_] * P + p_[m_]
            sb = s[m_]
            ia = np.full(P * db, ZROW_A, np.int64)
            ib = np.full(P * db, ZROW_B, np.int64)
            inA = sb < PIECE
            ia[j[inA]] = sb[inA]
            ib[j[~inA]] = sb[~inA] - PIECE
            ixa[:, col0[blk] * 8 : col0[blk + 1] * 8] = _wrap_idx(ia)
            ixb[:, col0[blk] * 8 : col0[blk + 1] * 8] = _wrap_idx(ib)
            mk[p_[m_], col0[blk] + d[m_]] = 0.0
        cores.append({"order": orders[c], "ixa": ixa, "ixb": ixb, "mk": mk})
    return cores, Db, col0, ncols


def _prep(x, edge_index, batch, u, weights, cores, Db, col0, ncols):
    batch = np.asarray(batch)
    maps = []
    for c in range(NCORES):
        pc = cores[c]
        order = pc["order"]
        m = {"ixa": pc["ixa"], "ixb": pc["ixb"]}
        # mask replicated over heads: [P, ncols*8] f32, layout (d, h)
        m["mb"] = _f32(np.repeat(pc["mk"], H, axis=1))
        xs = np.zeros((CAP, x.shape[1]), np.float32)
        xs[: NPC] = x[order]
        m["xT"] = _bf(xs.T)
        gs = np.zeros((CAP, G), np.float32)
        gs[np.arange(NPC), batch[order]] = 1.0
        m["gsel"] = _bf(gs)
        maps.append(m)

    counts = np.bincount(batch, minlength=G).astype(np.float32)
    vm = np.ones((P, 1), np.float32)
    vm[NPC - (NBLK - 1) * P :] = 0.0  # slots 106..127 of the last block
    shared = {
        "Wl1": _bf(weights["Wl1"]), "Wr1": _bf(weights["Wr1"]),
        "Wl2": _bf(weights["Wl2"]), "Wr2": _bf(weights["Wr2"]),
        "att1r": _bf(np.broadcast_to(weights["att1"].reshape(-1), (P, HC))),
        "att2r": _bf(np.broadcast_to(weights["att2"].reshape(-1), (P, HC))),
        "b1r": _bf(np.broadcast_to(weights["b1"], (P, HC))),
        "b2r": _bf(np.broadcast_to(weights["b2"], (P, HC))),
        "ident": _bf(np.eye(P, dtype=np.float32)),
        "vmask": _f32(vm),
        "crecip": _f32((1.0 / np.maximum(counts, 1.0)).reshape(G, 1)),
        "Wlin1": _bf(weights["W_lin1"]),
        "blin1r": _f32(np.broadcast_to(weights["b_lin1"], (G, 64))),
        "Wout": _bf(weights["W_out"]),
        "boutr": _f32(np.full((G, 1), float(weights["b_out"][0]), np.float32)),
        "ub": _bf(u),
    }
    for m in maps:
        m.update(shared)
    return maps


# ---------------------------------------------------------------------------
# Device program
# ---------------------------------------------------------------------------


def _build(Db, col0, ncols, in_dim=3):
    dt = mybir.dt
    bf = dt.bfloat16
    f32 = dt.float32
    nc = bacc.Bacc(None, num_swdge_queues=NQ)
    groups = [list(range(NCORES))]
    A_ = mybir.AluOpType
    AF = mybir.ActivationFunctionType

    def prm(name, shape, dtype):
        return nc.declare_dram_parameter(name, list(shape), dtype, isOutput=False)

    xT = prm("xT", [in_dim, CAP], bf)
    ixa = prm("ixa", [P, 8 * ncols], dt.int16)
    ixb = prm("ixb", [P, 8 * ncols], dt.int16)
    mbp = prm("mb", [P, 8 * ncols], f32)
    Wl1p = prm("Wl1", [in_dim, HC], bf)
    Wr1p = prm("Wr1", [in_dim, HC], bf)
    Wl2p = prm("Wl2", [HC, HC], bf)
    Wr2p = prm("Wr2", [HC, HC], bf)
    att1r = prm("att1r", [P, HC], bf)
    att2r = prm("att2r", [P, HC], bf)
    b1r = prm("b1r", [P, HC], bf)
    b2r = prm("b2r", [P, HC], bf)
    identp = prm("ident", [P, P], bf)
    vmaskp = prm("vmask", [P, 1], f32)
    gselp = prm("gsel", [CAP, G], bf)
    crecip = prm("crecip", [G, 1], f32)
    Wlin1 = prm("Wlin1", [HC, 64], bf)
    blin1r = prm("blin1r", [G, 64], f32)
    Woutp = prm("Wout", [64 + 3, 1], bf)
    boutr = prm("boutr", [G, 1], f32)
    ub = prm("ub", [G, 3], bf)
    out_g = nc.declare_dram_parameter("out_g", [G, 1], f32, isOutput=True)

    qctr = [0]

    def next_q():
        q = qctr[0] % NQ
        qctr[0] += 1
        return q

    with tile.TileContext(nc) as tc:
        with (
            tc.tile_pool(name="const", bufs=1) as constp,
            tc.tile_pool(name="meta", bufs=4) as metap,
            tc.tile_pool(name="gbuf", bufs=6) as gbufp,
            tc.tile_pool(name="work", bufs=2) as workp,
            tc.tile_pool(name="small", bufs=2) as smallp,
            tc.tile_pool(name="psA", bufs=2, space="PSUM") as psA,
            tc.tile_pool(name="psB", bufs=2, space="PSUM") as psB,
            tc.tile_pool(name="psG", bufs=1, space="PSUM") as psG,
            tc.tile_pool(name="dram", bufs=1, space="DRAM") as dram,
        ):
            # ---- constants to SBUF ----
            def cload(p):
                t = constp.tile([p.shape[0], p.shape[1]], p.dtype, name=p.name + "_s")
                nc.sync.dma_start(out=t[:], in_=p[:])
                return t

            def cload_k(p):
                nk = (p.shape[0] + P - 1) // P
                out = []
                for kt in range(nk):
                    rows = slice(kt * P, min((kt + 1) * P, p.shape[0]))
                    t = constp.tile(
                        [rows.stop - rows.start, p.shape[1]], p.dtype,
                        name=f"{p.name}_s{kt}",
                    )
                    nc.sync.dma_start(out=t[:], in_=p[rows, :])
                    out.append(t)
                return out

            xT_s = cload(xT)
            Wl1_s = cload_k(Wl1p)
            Wr1_s = cload_k(Wr1p)
            Wl2_s = cload_k(Wl2p)
            Wr2_s = cload_k(Wr2p)
            att1r_s = cload(att1r)
            att2r_s = cload(att2r)
            b1r_s = cload(b1r)
            b2r_s = cload(b2r)
            ident_s = cload(identp)
            vmask_s = cload(vmaskp)
            crecip_s = cload(crecip)
            Wlin1_s = cload_k(Wlin1)
            blin1r_s = cload(blin1r)
            Wout_s = cload(Woutp)
            boutr_s = cload(boutr)
            ub_s = cload(ub)

            # ---- internal DRAM ----
            xl1_own = dram.tile([CAP, HC], bf)
            xr1_tab = dram.tile([CAP, HC], bf)
            xl1_ext = dram.tile([CAPEXT, HC], bf, addr_space="Shared")
            xl1_priv = dram.tile([CAPEXT, HC], bf)
            xl2_own = dram.tile([CAP, HC], bf)
            xr2_tab = dram.tile([CAP, HC], bf)
            xl2_ext = dram.tile([CAPEXT, HC], bf, addr_space="Shared")
            xl2_priv = dram.tile([CAPEXT, HC], bf)
            gp_in = dram.tile([G, HC], f32)
            gp_out = dram.tile([G, HC], f32, addr_space="Shared")

            # ================= node tables =================
            def node_tables(lhsT_tiles, Wl_s, Wr_s, br_s, xl_dst, xr_dst, xrb_dst):
                for b in range(NBLK):
                    rows = slice(b * P, (b + 1) * P)
                    for W_s, tab, extra in ((Wl_s, xl_dst, None), (Wr_s, xr_dst, xrb_dst)):
                        ps = psA.tile([P, HC], f32, tag="a")
                        lts = lhsT_tiles(b)
                        assert len(lts) == len(W_s)
                        for i, lt in enumerate(lts):
                            nc.tensor.matmul(
                                ps[:], lhsT=lt, rhs=W_s[i][:],
                                start=(i == 0), stop=(i == len(lts) - 1),
                            )
                        ev = smallp.tile([P, HC], bf, tag="tabev")
                        nc.scalar.activation(out=ev[:], in_=ps[:], func=AF.Copy)
                        nc.sync.dma_start(out=tab[rows, :], in_=ev[:])
                        if extra is not None:
                            xb = smallp.tile([P, HC], bf, tag="tabxb")
                            nc.vector.tensor_tensor(
                                out=xb[:], in0=br_s[:], in1=ps[:], op=A_.subtract
                            )
                            nc.sync.dma_start(out=extra[rows, :], in_=xb[:])

            node_tables(
                lambda b: [xT_s[:, b * P : (b + 1) * P]],
                Wl1_s, Wr1_s, b1r_s, xl1_own, xr1_tab, None,
            )
            nc.gpsimd.collective_compute(
                "AllGather", A_.bypass, replica_groups=groups,
                ins=[xl1_own.opt()], outs=[xl1_ext.opt()],
            )
            for ch, eng in enumerate((nc.sync, nc.scalar, nc.sync, nc.scalar)):
                eng.dma_start(
                    out=xl1_priv[ch * (CAPEXT // 4) : (ch + 1) * (CAPEXT // 4), :],
                    in_=xl1_ext[ch * (CAPEXT // 4) : (ch + 1) * (CAPEXT // 4), :],
                )

            # ================= edge pipeline (slot-major) =================
            SW = 8  # d-ranks per subrectangle (1024-row gather call per piece)

            def tree(src, db_sub, width, tag):
                """Halve src [P, db_sub, width] over d down to a [P, 2, width]
                root (db_sub is even); returns (root2, leftover (tile,pos) list)."""
                cur = db_sub
                cur_t = src
                leftovers = []
                while True:
                    if cur % 2:
                        leftovers.append((cur_t, cur - 1))
                        cur -= 1
                    if cur <= 2:
                        break
                    h_ = cur // 2
                    dst_t = workp.tile([P, h_, width], bf, tag=f"{tag}{h_}")
                    nc.vector.tensor_tensor(
                        out=dst_t[:], in0=cur_t[:, 0:h_, :],
                        in1=cur_t[:, h_:cur, :], op=A_.add,
                    )
                    cur_t = dst_t
                    cur = h_
                return cur_t, leftovers

            def edge_layer(xl_ext, xl_shared, xr_tab, br_s, attr_s, layer):
                gpool_ps = None
                if layer == 2:
                    gpool_ps = psG.tile([G, HC], f32, name=f"gpool_ps{layer}")

                subs = []
                for b in range(NBLK):
                    db = int(Db[b])
                    for s0 in range(0, db, SW):
                        subs.append((b, s0, min(s0 + SW, db)))
                K_ = len(subs)
                st = [dict() for _ in range(K_)]
                blkmeta = {}
                blkacc = {}

                def stage_g(k):
                    b, s0, s1 = subs[k]
                    if s0 == 0:
                        bm = {}
                        db = int(Db[b])
                        c0 = int(col0[b])
                        rows = slice(b * P, (b + 1) * P)
                        xr_t = metap.tile([P, HC], bf, tag="xrb_l")
                        nc.sync.dma_start(out=xr_t[:], in_=xr_tab[rows, :])
                        mb_t = metap.tile([P, db * 8], f32, tag="mb")
                        nc.sync.dma_start(
                            out=mb_t[:], in_=mbp[:, c0 * 8 : (c0 + db) * 8]
                        )
                        ixa_t = metap.tile([P, db * 8], dt.int16, tag="ixa")
                        nc.sync.dma_start(
                            out=ixa_t[:], in_=ixa[:, c0 * 8 : (c0 + db) * 8]
                        )
                        ixb_t = metap.tile([P, db * 8], dt.int16, tag="ixb")
                        nc.sync.dma_start(
                            out=ixb_t[:], in_=ixb[:, c0 * 8 : (c0 + db) * 8]
                        )
                        bm["xr"], bm["mb"], bm["ixa"], bm["ixb"] = xr_t, mb_t, ixa_t, ixb_t
                        blkmeta[b] = bm
                    bm = blkmeta[b]
                    ds = s1 - s0
                    gA = gbufp.tile([P, ds, HC], bf, tag="gA")
                    gB = gbufp.tile([P, ds, HC], bf, tag="gB")
                    tbl = xl_shared if k < 8 else xl_ext
                    for dst_t, table, idxt in (
                        (gA, tbl[0:PIECE, :], bm["ixa"]),
                        (gB, tbl[PIECE:CAPEXT, :], bm["ixb"]),
                    ):
                        for q0 in range(s0, s1, DCH):
                            q1 = min(q0 + DCH, s1)
                            nc.gpsimd.dma_gather(
                                out_ap=dst_t[:, q0 - s0 : q1 - s0, :],
                                in_ap=table,
                                idxs_ap=idxt[:, q0 * 8 : q1 * 8],
                                num_idxs=(q1 - q0) * P,
                                num_idxs_reg=(q1 - q0) * P,
                                elem_size=HC,
                                queue_num=next_q(),
                            )
                    st[k]["gA"], st[k]["gB"] = gA, gB

                def stage_s1(k):
                    b, s0, s1 = subs[k]
                    ds = s1 - s0
                    u1 = workp.tile([P, ds, HC], bf, tag="u1")
                    nc.vector.tensor_tensor(
                        out=u1[:], in0=st[k]["gA"][:], in1=st[k]["gB"][:], op=A_.add
                    )
                    ut = workp.tile([P, ds, HC], bf, tag="ut")
                    nc.vector.tensor_tensor(
                        out=ut[:], in0=u1[:],
                        in1=blkmeta[b]["xr"][:].unsqueeze(1).broadcast_to([P, ds, HC]),
                        op=A_.add,
                    )
                    st[k]["u1"], st[k]["ut"] = u1, ut

                def stage_a1(k):
                    b, s0, s1 = subs[k]
                    ds = s1 - s0
                    ft = workp.tile([P, ds, HC], bf, tag="ft")
                    nc.scalar.activation(
                        out=ft[:], in_=st[k]["ut"][:], func=AF.Prelu, alpha=0.2
                    )
                    st[k]["ft"] = ft

                def stage_s2(k):
                    b, s0, s1 = subs[k]
                    ds = s1 - s0
                    dh = ds * H
                    Pt = workp.tile([P, ds, HC], bf, tag="Pt")
                    nc.vector.tensor_tensor(
                        out=Pt[:], in0=st[k]["ft"][:],
                        in1=attr_s[:].unsqueeze(1).broadcast_to([P, ds, HC]),
                        op=A_.mult,
                    )
                    lg = smallp.tile([P, dh], f32, tag="lg")
                    nc.vector.tensor_reduce(
                        out=lg[:],
                        in_=Pt[:].rearrange("p d (h c) -> p (d h) c", c=C),
                        axis=mybir.AxisListType.X, op=A_.add,
                    )
                    lgm = smallp.tile([P, dh], f32, tag="lgm")
                    nc.vector.tensor_tensor(
                        out=lgm[:], in0=lg[:],
                        in1=blkmeta[b]["mb"][:, s0 * 8 : s1 * 8], op=A_.add,
                    )
                    st[k]["lgm"] = lgm

                def stage_a2(k):
                    b, s0, s1 = subs[k]
                    ds = s1 - s0
                    dh = ds * H
                    exC = workp.tile([P, ds, HC], bf, tag="exC")
                    nc.scalar.activation(
                        out=exC[:].rearrange("p d (h c) -> p (d h) c", c=C),
                        in_=st[k]["lgm"][:].to_broadcast([P, dh, C]),
                        func=AF.Exp,
                    )
                    exs = smallp.tile([P, ds, H], bf, tag="exs")
                    nc.scalar.activation(
                        out=exs[:].rearrange("p d h -> p (d h)"),
                        in_=st[k]["lgm"][:], func=AF.Exp,
                    )
                    st[k]["exC"], st[k]["exs"] = exC, exs

                def accum(acc, root2_lo, width, tag):
                    root2, leftovers = root2_lo
                    views = [root2[:, 0, :], root2[:, 1, :]] + [
                        t_[:, pos, :] for t_, pos in leftovers
                    ]
                    if acc is None:
                        nt = smallp.tile([P, width], f32, tag=f"{tag}0")
                        nc.vector.tensor_tensor(
                            out=nt[:], in0=views[0], in1=views[1], op=A_.add
                        )
                        acc = nt
                        views = views[2:]
                    for v in views:
                        nt = smallp.tile([P, width], f32, tag=f"{tag}x")
                        nc.vector.tensor_tensor(out=nt[:], in0=acc[:], in1=v, op=A_.add)
                        acc = nt
                    return acc

                def finalize(b):
                    rows = slice(b * P, (b + 1) * P)
                    S, dn = blkacc[b]
                    dnm = smallp.tile([P, H], f32, tag="dnm")
                    nc.vector.tensor_scalar(
                        out=dnm[:], in0=dn[:], scalar1=1e-20, scalar2=None, op0=A_.max
                    )
                    rec = smallp.tile([P, H], f32, tag="rec")
                    nc.vector.reciprocal(out=rec[:], in_=dnm[:])
                    hsc = smallp.tile([P, HC], bf, tag="hsc")
                    nc.vector.tensor_tensor(
                        out=hsc[:].rearrange("p (h c) -> p h c", h=H),
                        in0=S[:].rearrange("p (h c) -> p h c", h=H),
                        in1=rec[:].unsqueeze(2).broadcast_to([P, H, C]),
                        op=A_.mult,
                    )
                    hfin = smallp.tile([P, HC], bf, tag="hfin")
                    nc.vector.tensor_tensor(
                        out=hfin[:], in0=hsc[:], in1=br_s[:], op=A_.add
                    )
                    hout = smallp.tile([P, HC], bf, tag="hout")
                    nc.scalar.activation(out=hout[:], in_=hfin[:], func=AF.Relu)
                    if b == NBLK - 1:
                        hvm = smallp.tile([P, HC], bf, tag="hvm")
                        nc.vector.tensor_tensor(
                            out=hvm[:], in0=hout[:],
                            in1=vmask_s[:].to_broadcast([P, HC]), op=A_.mult,
                        )
                        hout = hvm
                    if layer == 1:
                        tps_l = []
                        for kt in range(2):
                            tp = psA.tile([P, P], bf, tag="a")
                            nc.tensor.transpose(
                                out=tp[:], in_=hout[:, kt * P : (kt + 1) * P],
                                identity=ident_s[:],
                            )
                            tps = smallp.tile([P, P], bf, tag=f"htps{kt}")
                            nc.scalar.activation(out=tps[:], in_=tp[:], func=AF.Copy)
                            tps_l.append(tps)
                        # layer-2 node tables for this block, inline (PE idle here)
                        for W_s, tab in ((Wl2_s, xl2_own), (Wr2_s, xr2_tab)):
                            ps2 = psA.tile([P, HC], f32, tag="a")
                            for kt in range(2):
                                nc.tensor.matmul(
                                    ps2[:], lhsT=tps_l[kt][:], rhs=W_s[kt][:],
                                    start=(kt == 0), stop=(kt == 1),
                                )
                            ev2 = smallp.tile([P, HC], bf, tag="tabev")
                            nc.scalar.activation(out=ev2[:], in_=ps2[:], func=AF.Copy)
                            nc.sync.dma_start(out=tab[rows, :], in_=ev2[:])
                    else:
                        gsel_blk = metap.tile([P, G], bf, tag="gselb")
                        nc.sync.dma_start(out=gsel_blk[:], in_=gselp[rows, :])
                        nc.tensor.matmul(
                            gpool_ps[:], lhsT=gsel_blk[:], rhs=hout[:],
                            start=(b == 0), stop=(b == NBLK - 1),
                        )

                def stage_s3(k):
                    b, s0, s1 = subs[k]
                    ds = s1 - s0
                    msg = workp.tile([P, ds, HC], bf, tag="msg")
                    nc.vector.tensor_tensor(
                        out=msg[:], in0=st[k]["u1"][:], in1=st[k]["exC"][:], op=A_.mult
                    )
                    Sa, da = blkacc.get(b, (None, None))
                    Sa = accum(Sa, tree(msg, ds, HC, "tm"), HC, "Sa")
                    da = accum(da, tree(st[k]["exs"], ds, H, "td"), H, "da")
                    blkacc[b] = (Sa, da)
                    if s1 == int(Db[b]):
                        finalize(b)
                    st[k].clear()

                # skewed pipeline: gathers lead by 5; DVE interleaves 3 stages
                GLEAD = 5
                for kk in range(min(GLEAD, K_)):
                    stage_g(kk)
                for k in range(K_):
                    if k + GLEAD < K_:
                        stage_g(k + GLEAD)
                    stage_s1(k)
                    stage_a1(k)
                    if k >= 1:
                        stage_s2(k - 1)
                        stage_a2(k - 1)
                    if k >= 2:
                        stage_s3(k - 2)
                stage_s2(K_ - 1)
                stage_a2(K_ - 1)
                if K_ >= 2:
                    stage_s3(K_ - 2)
                stage_s3(K_ - 1)
                return gpool_ps

            edge_layer(xl1_priv, xl1_ext, xr1_tab, b1r_s, att1r_s, layer=1)

            # layer-2 node tables were built inline during layer 1
            nc.gpsimd.collective_compute(
                "AllGather", A_.bypass, replica_groups=groups,
                ins=[xl2_own.opt()], outs=[xl2_ext.opt()],
            )
            for ch, eng in enumerate((nc.sync, nc.scalar, nc.sync, nc.scalar)):
                eng.dma_start(
                    out=xl2_priv[ch * (CAPEXT // 4) : (ch + 1) * (CAPEXT // 4), :],
                    in_=xl2_ext[ch * (CAPEXT // 4) : (ch + 1) * (CAPEXT // 4), :],
                )

            gpool_ps = edge_layer(xl2_priv, xl2_ext, xr2_tab, b2r_s, att2r_s, layer=2)

            # ================= pool + MLP =================
            gsum = smallp.tile([G, HC], f32, tag="gsum")
            nc.scalar.activation(out=gsum[:], in_=gpool_ps[:], func=AF.Copy)
            nc.sync.dma_start(out=gp_in[:], in_=gsum[:])
            nc.gpsimd.collective_compute(
                "AllReduce", A_.add, replica_groups=groups,
                ins=[gp_in.opt()], outs=[gp_out.opt()],
            )
            gsum2 = smallp.tile([G, HC], f32, tag="gsum2")
            nc.sync.dma_start(out=gsum2[:], in_=gp_out[:])
            gmean = smallp.tile([G, HC], bf, tag="gmean")
            nc.vector.tensor_scalar(
                out=gmean[:], in0=gsum2[:], scalar1=crecip_s[:, 0:1], scalar2=None,
                op0=A_.mult,
            )
            gT = []
            for kt in range(2):
                tp = psA.tile([P, G], bf, tag="a")
                nc.tensor.transpose(
                    out=tp[:], in_=gmean[:, kt * P : (kt + 1) * P], identity=ident_s[:]
                )
                gkt = smallp.tile([P, G], bf, tag="gT", name=f"gT{kt}")
                nc.scalar.activation(out=gkt[:], in_=tp[:], func=AF.Copy)
                gT.append(gkt)
            lin_ps = psB.tile([G, 64], f32, tag="b")
            for kt in range(2):
                nc.tensor.matmul(
                    lin_ps[:], lhsT=gT[kt][:], rhs=Wlin1_s[kt][:],
                    start=(kt == 0), stop=(kt == 1),
                )
            lin = smallp.tile([G, 64], f32, tag="lin")
            nc.vector.tensor_tensor(out=lin[:], in0=lin_ps[:], in1=blin1r_s[:], op=A_.add)
            glu = smallp.tile([G, P], bf, tag="glu")
            nc.scalar.activation(out=glu[:, 0:64], in_=lin[:], func=AF.Relu)
            nc.vector.tensor_copy(out=glu[:, 64:67], in_=ub_s[:])
            nc.gpsimd.memset(glu[:, 67:P], 0.0)
            tp = psA.tile([P, G], bf, tag="a")
            nc.tensor.transpose(out=tp[:], in_=glu[:], identity=ident_s[:])
            gluT = smallp.tile([P, G], bf, tag="gluT")
            nc.scalar.activation(out=gluT[:], in_=tp[:], func=AF.Copy)
            out_ps = psB.tile([G, 1], f32, tag="b")
            nc.tensor.matmul(
                out_ps[:], lhsT=gluT[0:67, :], rhs=Wout_s[:], start=True, stop=True
            )
            outs = smallp.tile([G, 1], f32, tag="outs")
            nc.vector.tensor_tensor(out=outs[:], in0=out_ps[:], in1=boutr_s[:], op=A_.add)
            nc.sync.dma_start(out=out_g[:], in_=outs[:])

    nc.compile()
    _split_waits(nc)
    return nc


# ---------------------------------------------------------------------------
# Entry point
# ---------------------------------------------------------------------------


def kernel(**inputs):
    import os

    from concourse.bass_utils import run_bass_kernel_spmd

    x = np.asarray(inputs["x"], np.float32)
    edge_index = np.asarray(inputs["edge_index"], np.int64)
    batch = np.asarray(inputs["batch"], np.int64)
    u = np.asarray(inputs["u"], np.float32)
    weights = {
        k: np.asarray(inputs[k], np.float32)
        for k in ("Wl1", "Wr1", "att1", "b1", "Wl2", "Wr2", "att2", "b2",
                  "W_lin1", "b_lin1", "W_out", "b_out")
    }
    cores, Db, col0, ncols = _plan(edge_index)
    maps = _prep(x, edge_index, batch, u, weights, cores, Db, col0, ncols)
    nc = _build(Db, col0, ncols, in_dim=x.shape[1])
    trace = bool(os.environ.get("KERNEL_TRACE"))
    kw = {"trace": trace}
    if trace and os.environ.get("KERNEL_TRACE_DIR"):
        kw["tmpdir"] = os.environ["KERNEL_TRACE_DIR"]
    try:
        res = run_bass_kernel_spmd(nc, maps, list(range(NCORES)), **kw)
    except ModuleNotFoundError:
        res = run_bass_kernel_spmd(nc, maps, list(range(NCORES)))
    if trace and getattr(res, "exec_time_ns", None) is not None:
        print(f"HW exec time: {res.exec_time_ns} ns")
    return res.results[0]["out_g"].reshape(G).astype(np.float32)
